# revision 11
# baseline (speedup 1.0000x reference)
"""2-layer GCN (PyG GCNConv semantics) on 8 Trainium2 NeuronCores.

Computation (matches the jax reference):
    src,dst = add_self_loops(edge_index)
    h1 = relu(gcn_conv(x, W1, b1));  h2 = gcn_conv(h1, W2, b2)
    out = log_softmax(h2 @ Wl + bl, axis=1)
where gcn_conv(x,W,b) = D^-1/2 (A+I) D^-1/2 (x@W) + b.

Device strategy (graph/data parallel, nodes partitioned across 8 cores):
  - The symmetric normalization factorizes: pre-scale each node's transformed
    features by dinv[node] (folded into the transform's PSUM->SBUF copy),
    aggregate un-scaled, post-scale rows by dinv[dst] on copy-out.  The
    self-loop term never goes through the edge machinery: it is added
    densely on copy-out (h = dinv * (agg + t_local) + b).
  - Per layer, each core computes its local shard of the scaled feature
    table T = dinv * (h @ W), then an AllGather replicates T to all cores.
  - Aggregation: edges are pre-sorted by (dst window, src half) on the host
    and packed into 128-slot chunks per 128-destination window.  GPSIMD
    dma_gather instructions fetch T[src] rows (edge i of a window ->
    partition i%128, chunk i//128; int16 indices, so the node table is
    split into lo/hi halves gathered by separate instructions; max 1024
    indices per instruction -- a hardware limit -- spread round-robin over
    4 SWDGE queues so descriptor generation pipelines with the transfers),
    a DVE is_equal-vs-iota generates the per-chunk one-hot selection
    matrix, and a PSUM-accumulated TensorE matmul chain reduces slots into
    the window's [128 dst, F] accumulator.  Pad slots point at row 0 and
    carry an all-zero one-hot column, so they contribute nothing.
"""

import numpy as np
import ml_dtypes

import concourse.bass as bass
import concourse.mybir as mybir
import concourse.tile as tile
from concourse import bacc

P = 128
N_CORES = 8
F_IN, F_HID, F_OUT = 512, 128, 64
KIN = F_IN // P  # fin chunks
MAXCH = 8        # chunks per dma_gather (1024-index HW limit)
NQ = 4           # SWDGE queues

_BF16 = mybir.dt.bfloat16
_F32 = mybir.dt.float32
_I16 = mybir.dt.int16

_PROGRAM_CACHE = {}


def build_program(W, CAP_LO, CAP_HI, debug_taps=False):
    """Build the SPMD Tile program. W = windows/core; CAP_* = chunks/window
    gathered from the lo/hi half of the node table."""
    CAP = CAP_LO + CAP_HI
    nc = bacc.Bacc("TRN2", target_bir_lowering=False, debug=False,
                   num_devices=N_CORES, num_swdge_queues=NQ)
    n_loc = W * P
    n_pad = n_loc * N_CORES
    half = n_pad // 2

    def inp(name, shape, dt):
        return nc.dram_tensor(name, shape, dt, kind="ExternalInput").ap()

    x_in = inp("x", [n_loc, F_IN], _F32)
    idx_in = inp("idx", [P, W, CAP * 8], _I16)
    dst_in = inp("dst", [P, W * CAP, 1], _F32)
    dinv_in = inp("dinv", [P, W], _F32)
    w1_in = inp("w1", [P, KIN * F_HID], _BF16)   # [p, k*F_HID+j] = W1[k*128+p, j]
    w2_in = inp("w2", [P, F_OUT], _BF16)
    wl_in = inp("wl", [F_OUT, F_OUT], _F32)
    b1_in = inp("b1r", [P, F_HID], _F32)         # bias replicated across partitions
    b2_in = inp("b2r", [P, F_OUT], _F32)
    bl_in = inp("blr", [P, F_OUT], _F32)
    iota_in = inp("iota", [P, 1, P], _BF16)      # iota_in[p,0,j] = j
    idf_in = inp("idf", [P, P], _F32)            # identity f32
    idb_in = inp("idb", [P, P], _BF16)           # identity bf16
    out_ext = nc.dram_tensor("out", [n_loc, F_OUT], _F32,
                             kind="ExternalOutput").ap()

    taps = {}
    if debug_taps:
        taps["h10"] = nc.dram_tensor("tap_h10", [P, F_HID], _F32,
                                     kind="ExternalOutput").ap()

    rr = [0]  # SWDGE queue round-robin

    with tile.TileContext(nc) as tc:
        with tc.tile_pool(name="const", bufs=1) as cp, \
             tc.tile_pool(name="work", bufs=3) as wp, \
             tc.tile_pool(name="gp", bufs=16) as gp, \
             tc.tile_pool(name="ps", bufs=2, space="PSUM") as ps, \
             tc.tile_pool(name="dram", bufs=1, space="DRAM") as dp:

            # ---- resident constants / metadata ----
            idx_sb = cp.tile([P, W, CAP * 8], _I16)
            dst_sb = cp.tile([P, W * CAP, 1], _F32)
            dinv_sb = cp.tile([P, W], _F32)
            w1_sb = cp.tile([P, KIN * F_HID], _BF16)
            w2_sb = cp.tile([P, F_OUT], _BF16)
            wl_sb = cp.tile([F_OUT, F_OUT], _F32)
            b1_sb = cp.tile([P, F_HID], _F32)
            b2_sb = cp.tile([P, F_OUT], _F32)
            bl_sb = cp.tile([P, F_OUT], _F32)
            iota_sb = cp.tile([P, 1, P], _BF16)
            idf_sb = cp.tile([P, P], _F32)
            idb_sb = cp.tile([P, P], _BF16)
            h1T_sb = cp.tile([P, W * P], _BF16)    # h1 transposed, feature-major
            t1res_sb = cp.tile([P, W * F_HID], _BF16)  # local T1 rows (self loops)
            t2res_sb = cp.tile([P, W * F_OUT], _BF16)  # local T2 rows (self loops)

            nc.sync.dma_start(out=idx_sb[:], in_=idx_in[:])
            nc.sync.dma_start(out=dst_sb[:], in_=dst_in[:])
            nc.sync.dma_start(out=dinv_sb[:], in_=dinv_in[:])
            nc.sync.dma_start(out=w1_sb[:], in_=w1_in[:])
            nc.sync.dma_start(out=w2_sb[:], in_=w2_in[:])
            nc.sync.dma_start(out=wl_sb[:], in_=wl_in[:])
            nc.sync.dma_start(out=b1_sb[:], in_=b1_in[:])
            nc.sync.dma_start(out=b2_sb[:], in_=b2_in[:])
            nc.sync.dma_start(out=bl_sb[:], in_=bl_in[:])
            nc.sync.dma_start(out=iota_sb[:], in_=iota_in[:])
            nc.sync.dma_start(out=idf_sb[:], in_=idf_in[:])
            nc.sync.dma_start(out=idb_sb[:], in_=idb_in[:])

            # ---- collective tables (rows are 128 bf16 = 256B for dma_gather;
            #      T2 uses cols 0:64, the rest is never read) ----
            t1_loc = dp.tile([n_loc, P], _BF16)
            t1_full = dp.tile([n_pad, P], _BF16)
            t2_loc = dp.tile([n_loc, P], _BF16)
            t2_full = dp.tile([n_pad, P], _BF16)

            def gather_window(table, w):
                """Issue the window's gathers; return list of (tile, ch0, n)."""
                tiles = []
                for ch0, capr, col0, lo in [(0, CAP_LO, 0, 0),
                                            (CAP_LO, CAP_HI, CAP_LO * 8, half)]:
                    for g0 in range(0, capr, MAXCH):
                        g1 = min(g0 + MAXCH, capr)
                        gt = gp.tile([P, MAXCH, P], _BF16, tag="g")
                        nc.gpsimd.dma_gather(
                            out_ap=gt[:, 0:g1 - g0, :],
                            in_ap=table[lo:lo + half, :],
                            idxs_ap=idx_sb[:, w, col0 + g0 * 8:col0 + g1 * 8],
                            num_idxs=(g1 - g0) * P, num_idxs_reg=(g1 - g0) * P,
                            elem_size=P, queue_num=rr[0] % NQ)
                        rr[0] += 1
                        tiles.append((gt, ch0 + g0, g1 - g0))
                return tiles

            def one_hot(w):
                # Per-chunk tensor_scalar (contiguous input + per-partition
                # scalar) -- broadcast-AP tensor_tensor is ~10x slower on DVE.
                S = wp.tile([P, CAP, P], _BF16, tag="S")
                for c in range(CAP):
                    nc.vector.tensor_scalar(
                        out=S[:, c, :], in0=iota_sb[:, 0, :],
                        scalar1=dst_sb[:, w * CAP + c, :], scalar2=None,
                        op0=mybir.AluOpType.is_equal)
                return S

            def reduce_window(tiles, S, fdim):
                wps = ps.tile([P, fdim], _F32, tag="wp")
                n_mm = sum(n for _, _, n in tiles)
                i = 0
                for gt, ch0, n in tiles:
                    for c in range(n):
                        nc.tensor.matmul(out=wps[:], lhsT=S[:, ch0 + c, :],
                                         rhs=gt[:, c, 0:fdim],
                                         start=(i == 0), stop=(i == n_mm - 1))
                        i += 1
                return wps

            # ---- phase A: T1 = dinv * (x @ W1), local shard ----
            for w in range(W):
                x_t = wp.tile([P, F_IN], _F32, tag="xt")
                nc.sync.dma_start(out=x_t[:], in_=x_in[w * P:(w + 1) * P, :])
                x_bf = wp.tile([P, F_IN], _BF16, tag="xbf")
                nc.vector.tensor_copy(out=x_bf[:], in_=x_t[:])
                xT = wp.tile([P, KIN, P], _BF16, tag="xT")
                for k in range(KIN):
                    trp = ps.tile([P, P], _BF16, tag="tr")
                    nc.tensor.transpose(out=trp[:], in_=x_bf[:, k * P:(k + 1) * P],
                                        identity=idb_sb[:])
                    nc.vector.tensor_copy(out=xT[:, k, :], in_=trp[:])
                hp = ps.tile([P, F_HID], _F32, tag="mm")
                for k in range(KIN):
                    nc.tensor.matmul(out=hp[:], lhsT=xT[:, k, :],
                                     rhs=w1_sb[:, k * F_HID:(k + 1) * F_HID],
                                     start=(k == 0), stop=(k == KIN - 1))
                nc.vector.tensor_scalar(
                    out=t1res_sb[:, w * F_HID:(w + 1) * F_HID], in0=hp[:],
                    scalar1=dinv_sb[:, w:w + 1], scalar2=None,
                    op0=mybir.AluOpType.mult)
                nc.sync.dma_start(out=t1_loc[w * P:(w + 1) * P, :],
                                  in_=t1res_sb[:, w * F_HID:(w + 1) * F_HID])

            # ---- phase B: AllGather T1 ----
            nc.gpsimd.collective_compute(
                "AllGather", mybir.AluOpType.bypass,
                replica_groups=[list(range(N_CORES))],
                ins=[t1_loc.opt()], outs=[t1_full.opt()],
            )

            # ---- phase C: L1 aggregation + relu; build h1T ----
            for w in range(W):
                tiles = gather_window(t1_full, w)
                S = one_hot(w)
                wps = reduce_window(tiles, S, F_HID)
                h1_t = wp.tile([P, F_HID], _F32, tag="h1")
                # h1 = dinv*(agg + t1_local) + b1 ; then relu
                nc.vector.tensor_add(out=h1_t[:], in0=wps[:],
                                     in1=t1res_sb[:, w * F_HID:(w + 1) * F_HID])
                nc.vector.tensor_scalar(out=h1_t[:], in0=h1_t[:],
                                        scalar1=dinv_sb[:, w:w + 1], scalar2=None,
                                        op0=mybir.AluOpType.mult)
                nc.vector.tensor_add(out=h1_t[:], in0=h1_t[:], in1=b1_sb[:])
                h1_bf = wp.tile([P, F_HID], _BF16, tag="h1b")
                nc.scalar.activation(out=h1_bf[:], in_=h1_t[:],
                                     func=mybir.ActivationFunctionType.Relu)
                if debug_taps and w == 0:
                    nc.sync.dma_start(out=taps["h10"][:], in_=h1_t[:])
                trp = ps.tile([P, P], _BF16, tag="tr")
                nc.tensor.transpose(out=trp[:], in_=h1_bf[:], identity=idb_sb[:])
                nc.vector.tensor_copy(out=h1T_sb[:, w * P:(w + 1) * P], in_=trp[:])

            # ---- phase D: T2 = dinv * (h1 @ W2), local shard ----
            for w in range(W):
                t2p = ps.tile([P, F_OUT], _F32, tag="mm")
                nc.tensor.matmul(out=t2p[:], lhsT=h1T_sb[:, w * P:(w + 1) * P],
                                 rhs=w2_sb[:], start=True, stop=True)
                nc.vector.tensor_scalar(
                    out=t2res_sb[:, w * F_OUT:(w + 1) * F_OUT], in0=t2p[:],
                    scalar1=dinv_sb[:, w:w + 1], scalar2=None,
                    op0=mybir.AluOpType.mult)
                t2_t = wp.tile([P, P], _BF16, tag="t2")
                nc.vector.memset(t2_t[:, F_OUT:P], 0.0)
                nc.vector.tensor_copy(out=t2_t[:, 0:F_OUT],
                                      in_=t2res_sb[:, w * F_OUT:(w + 1) * F_OUT])
                nc.sync.dma_start(out=t2_loc[w * P:(w + 1) * P, :], in_=t2_t[:])

            # ---- phase E: AllGather T2 ----
            nc.gpsimd.collective_compute(
                "AllGather", mybir.AluOpType.bypass,
                replica_groups=[list(range(N_CORES))],
                ins=[t2_loc.opt()], outs=[t2_full.opt()],
            )

            # ---- phase F: L2 aggregation + final linear + log_softmax ----
            for w in range(W):
                tiles = gather_window(t2_full, w)
                S = one_hot(w)
                wps = reduce_window(tiles, S, F_OUT)
                h2_t = wp.tile([P, F_OUT], _F32, tag="h2")
                nc.vector.tensor_add(out=h2_t[:], in0=wps[:],
                                     in1=t2res_sb[:, w * F_OUT:(w + 1) * F_OUT])
                nc.vector.tensor_scalar(out=h2_t[:], in0=h2_t[:],
                                        scalar1=dinv_sb[:, w:w + 1], scalar2=None,
                                        op0=mybir.AluOpType.mult)
                nc.vector.tensor_add(out=h2_t[:], in0=h2_t[:], in1=b2_sb[:])
                # final linear: logits = h2 @ Wl + bl (f32)
                h2tp = ps.tile([F_OUT, P], _F32, tag="tr")
                nc.tensor.transpose(out=h2tp[:], in_=h2_t[:], identity=idf_sb[:])
                h2T = wp.tile([F_OUT, P], _F32, tag="h2T")
                nc.vector.tensor_copy(out=h2T[:], in_=h2tp[:])
                lp = ps.tile([P, F_OUT], _F32, tag="mm")
                nc.tensor.matmul(out=lp[:], lhsT=h2T[:], rhs=wl_sb[:],
                                 start=True, stop=True)
                lg = wp.tile([P, F_OUT], _F32, tag="lg")
                nc.vector.tensor_add(out=lg[:], in0=lp[:], in1=bl_sb[:])
                # log_softmax over the free dim
                negmax = wp.tile([P, 1], _F32, tag="nm")
                nc.vector.tensor_reduce(out=negmax[:], in_=lg[:],
                                        axis=mybir.AxisListType.X,
                                        op=mybir.AluOpType.max, negate=True)
                ex = wp.tile([P, F_OUT], _F32, tag="ex")
                sm = wp.tile([P, 1], _F32, tag="sm")
                nc.scalar.activation(out=ex[:], in_=lg[:],
                                     func=mybir.ActivationFunctionType.Exp,
                                     bias=negmax[:], scale=1.0, accum_out=sm[:])
                ls = wp.tile([P, 1], _F32, tag="ls")
                nc.scalar.activation(out=ls[:], in_=sm[:],
                                     func=mybir.ActivationFunctionType.Ln)
                o_t = wp.tile([P, F_OUT], _F32, tag="ot")
                nc.vector.tensor_scalar(out=o_t[:], in0=lg[:],
                                        scalar1=negmax[:], scalar2=ls[:],
                                        op0=mybir.AluOpType.add,
                                        op1=mybir.AluOpType.subtract)
                nc.sync.dma_start(out=out_ext[w * P:(w + 1) * P, :], in_=o_t[:])

    nc.compile()
    return nc


def preprocess(x, edge_index, W1, b1, W2, b2, Wl, bl):
    """Host-side sharding: sort edges by (dst window, src half), pack chunks.
    Self-loops are NOT packed as edges; they are applied densely on-device."""
    n = x.shape[0]
    src = np.asarray(edge_index[0], dtype=np.int64)
    dst = np.asarray(edge_index[1], dtype=np.int64)

    deg = np.bincount(dst, minlength=n).astype(np.float64) + 1.0  # + self loop
    dinv = 1.0 / np.sqrt(deg)

    W = int(np.ceil(n / (N_CORES * P)))
    n_loc = W * P
    n_pad = n_loc * N_CORES
    half = n_pad // 2

    hi = (src >= half).astype(np.int64)
    order = np.argsort((dst // P) * 2 + hi, kind="stable")
    s_src = src[order]
    s_dst = dst[order]
    s_hi = hi[order]

    n_windows = N_CORES * W
    group = (s_dst // P) * 2 + s_hi
    g_counts = np.bincount(group, minlength=2 * n_windows)
    g_starts = np.concatenate([[0], np.cumsum(g_counts)[:-1]])
    j = np.arange(len(s_src)) - g_starts[group]   # rank within group

    CAP_LO = int(np.ceil(g_counts[0::2].max() / P))
    CAP_HI = int(np.ceil(g_counts[1::2].max() / P))
    CAP = CAP_LO + CAP_HI

    gw = s_dst // P
    core = gw // W
    lw = gw % W
    slot = j + s_hi * (CAP_LO * P)                # slot within the window
    p_slot = slot % P
    ch = slot // P

    # dma_gather index streams: index i of a region -> [16r + i%16, i//16]
    idx16 = np.where(s_hi == 1, s_src - half, s_src).astype(np.int16)
    idx_tmp = np.zeros((N_CORES, 16, W, CAP * 8), dtype=np.int16)  # pads -> row 0
    idx_tmp[core, j % 16, lw, s_hi * (CAP_LO * 8) + j // 16] = idx16
    idx_arr = np.tile(idx_tmp, (1, 8, 1, 1))

    dst_arr = np.full((N_CORES, P, W * CAP, 1), -1.0, dtype=np.float32)
    dst_arr[core, p_slot, lw * CAP + ch, 0] = (s_dst % P).astype(np.float32)

    x_pad = np.zeros((n_pad, F_IN), dtype=np.float32)
    x_pad[:n] = np.asarray(x, dtype=np.float32)
    dinv_pad = np.zeros(n_pad, dtype=np.float32)
    dinv_pad[:n] = dinv

    bf16 = ml_dtypes.bfloat16
    w1_c = np.ascontiguousarray(
        np.asarray(W1, np.float32).reshape(KIN, P, F_HID).transpose(1, 0, 2)
        .reshape(P, KIN * F_HID)).astype(bf16)
    w2_c = np.asarray(W2, np.float32).astype(bf16)
    wl_c = np.asarray(Wl, np.float32)
    b1_r = np.broadcast_to(np.asarray(b1, np.float32), (P, F_HID)).copy()
    b2_r = np.broadcast_to(np.asarray(b2, np.float32), (P, F_OUT)).copy()
    bl_r = np.broadcast_to(np.asarray(bl, np.float32), (P, F_OUT)).copy()
    iota = np.broadcast_to(np.arange(P, dtype=np.float32), (P, 1, P)).astype(bf16).copy()
    idf = np.eye(P, dtype=np.float32)
    idb = np.eye(P, dtype=np.float32).astype(bf16)

    in_maps = []
    for c in range(N_CORES):
        dv = dinv_pad[c * n_loc:(c + 1) * n_loc].reshape(W, P).T.copy()
        in_maps.append({
            "x": x_pad[c * n_loc:(c + 1) * n_loc],
            "idx": idx_arr[c],
            "dst": dst_arr[c],
            "dinv": np.ascontiguousarray(dv),
            "w1": w1_c, "w2": w2_c, "wl": wl_c,
            "b1r": b1_r, "b2r": b2_r, "blr": bl_r,
            "iota": iota, "idf": idf, "idb": idb,
        })
    return in_maps, (W, CAP_LO, CAP_HI), n, n_loc


def kernel(x, edge_index, W1, b1, W2, b2, Wl, bl):
    from concourse.bass_utils import run_bass_kernel_spmd

    in_maps, key, n, n_loc = preprocess(x, edge_index, W1, b1, W2, b2, Wl, bl)
    if key not in _PROGRAM_CACHE:
        _PROGRAM_CACHE[key] = build_program(*key)
    nc = _PROGRAM_CACHE[key]
    res = run_bass_kernel_spmd(nc, in_maps, list(range(N_CORES)))
    out = np.concatenate([res.results[c]["out"] for c in range(N_CORES)], axis=0)
    return out[:n].astype(np.float32)


# revision 15
# speedup vs baseline: 1.3103x; 1.3103x over previous
"""2-layer GCN (PyG GCNConv semantics) on 8 Trainium2 NeuronCores.

Computation (matches the jax reference):
    src,dst = add_self_loops(edge_index)
    h1 = relu(gcn_conv(x, W1, b1));  h2 = gcn_conv(h1, W2, b2)
    out = log_softmax(h2 @ Wl + bl, axis=1)
where gcn_conv(x,W,b) = D^-1/2 (A+I) D^-1/2 (x@W) + b.

Device strategy (graph/data parallel, nodes partitioned across 8 cores):
  - The symmetric normalization factorizes: pre-scale each node's transformed
    features by dinv[node] (folded into the transform's PSUM->SBUF copy),
    aggregate un-scaled, post-scale rows by dinv[dst] on copy-out.  The
    self-loop term never goes through the edge machinery: it is added
    densely on copy-out (h = dinv * (agg + t_local) + b).
  - Per layer, each core computes its local shard of the scaled feature
    table T = dinv * (h @ W), then an AllGather replicates T to all cores.
  - Aggregation: edges are pre-sorted by (dst window, src half) on the host
    and packed into 128-slot chunks per 128-destination window.  GPSIMD
    dma_gather instructions fetch T[src] rows (edge i of a window ->
    partition i%128, chunk i//128; int16 indices, so the node table is
    split into lo/hi halves gathered by separate instructions; max 1024
    indices per instruction -- a hardware limit -- spread round-robin over
    4 SWDGE queues so descriptor generation pipelines with the transfers),
    a DVE is_equal-vs-iota generates the per-chunk one-hot selection
    matrix, and a PSUM-accumulated TensorE matmul chain reduces slots into
    the window's [128 dst, F] accumulator.  Pad slots point at row 0 and
    carry an all-zero one-hot column, so they contribute nothing.
"""

import numpy as np
import ml_dtypes

import concourse.bass as bass
import concourse.mybir as mybir
import concourse.tile as tile
from concourse import bacc

P = 128
N_CORES = 8
F_IN, F_HID, F_OUT = 512, 128, 64
KIN = F_IN // P  # fin chunks
MAXCH = 8        # chunks per dma_gather (1024-index HW limit)
NQ = 4           # SWDGE queues

_BF16 = mybir.dt.bfloat16
_F32 = mybir.dt.float32
_I16 = mybir.dt.int16

_PROGRAM_CACHE = {}


def build_program(W, CAP_LO, CAP_HI, debug_taps=False):
    """Build the SPMD Tile program. W = windows/core; CAP_* = chunks/window
    gathered from the lo/hi half of the node table."""
    CAP = CAP_LO + CAP_HI
    nc = bacc.Bacc("TRN2", target_bir_lowering=False, debug=False,
                   num_devices=N_CORES, num_swdge_queues=NQ)
    n_loc = W * P
    n_pad = n_loc * N_CORES
    half = n_pad // 2

    def inp(name, shape, dt):
        return nc.dram_tensor(name, shape, dt, kind="ExternalInput").ap()

    x_in = inp("x", [n_loc, F_IN], _F32)
    idx_in = inp("idx", [P, W, CAP * 8], _I16)
    dst_in = inp("dst", [P, W * CAP, 1], _F32)
    dinv_in = inp("dinv", [P, W], _F32)
    w1_in = inp("w1", [P, KIN * F_HID], _BF16)   # [p, k*F_HID+j] = W1[k*128+p, j]
    w2_in = inp("w2", [P, F_OUT], _BF16)
    wl_in = inp("wl", [F_OUT, F_OUT], _F32)
    b1_in = inp("b1r", [P, F_HID], _F32)         # bias replicated across partitions
    b2_in = inp("b2r", [P, F_OUT], _F32)
    bl_in = inp("blr", [P, F_OUT], _F32)
    iota_in = inp("iota", [P, 1, P], _BF16)      # iota_in[p,0,j] = j
    iotar_in = inp("iotar", [P, CAP * P], _BF16)  # iota repeated CAP times
    cnt_in = inp("cnt", [1, W * 16], mybir.dt.int32)  # valid idx count per subgather
    idf_in = inp("idf", [P, P], _F32)            # identity f32
    idb_in = inp("idb", [P, P], _BF16)           # identity bf16
    out_ext = nc.dram_tensor("out", [n_loc, F_OUT], _F32,
                             kind="ExternalOutput").ap()

    taps = {}
    if debug_taps:
        taps["h10"] = nc.dram_tensor("tap_h10", [P, F_HID], _F32,
                                     kind="ExternalOutput").ap()

    rr = [0]  # SWDGE queue round-robin
    cnt_regs = [nc.gpsimd.alloc_register(f"cntreg{i}") for i in range(8)]

    with tile.TileContext(nc) as tc:
        with tc.tile_pool(name="const", bufs=1) as cp, \
             tc.tile_pool(name="work", bufs=3) as wp, \
             tc.tile_pool(name="gp", bufs=16) as gp, \
             tc.tile_pool(name="ps", bufs=2, space="PSUM") as ps, \
             tc.tile_pool(name="dram", bufs=1, space="DRAM") as dp:

            # ---- resident constants / metadata ----
            idx_sb = cp.tile([P, W, CAP * 8], _I16)
            dst_sb = cp.tile([P, W * CAP, 1], _F32)
            dinv_sb = cp.tile([P, W], _F32)
            w1_sb = cp.tile([P, KIN * F_HID], _BF16)
            w2_sb = cp.tile([P, F_OUT], _BF16)
            wl_sb = cp.tile([F_OUT, F_OUT], _F32)
            b1_sb = cp.tile([P, F_HID], _F32)
            b2_sb = cp.tile([P, F_OUT], _F32)
            bl_sb = cp.tile([P, F_OUT], _F32)
            iota_sb = cp.tile([P, 1, P], _BF16)
            iotar_sb = cp.tile([P, CAP * P], _BF16)
            cnt_sb = cp.tile([1, W * 16], mybir.dt.int32)
            idf_sb = cp.tile([P, P], _F32)
            idb_sb = cp.tile([P, P], _BF16)
            h1T_sb = cp.tile([P, W * P], _BF16)    # h1 transposed, feature-major
            t1res_sb = cp.tile([P, W * F_HID], _BF16)  # local T1 rows (self loops)
            t2res_sb = cp.tile([P, W * F_OUT], _BF16)  # local T2 rows (self loops)

            nc.sync.dma_start(out=idx_sb[:], in_=idx_in[:])
            nc.sync.dma_start(out=dst_sb[:], in_=dst_in[:])
            nc.sync.dma_start(out=dinv_sb[:], in_=dinv_in[:])
            nc.sync.dma_start(out=w1_sb[:], in_=w1_in[:])
            nc.sync.dma_start(out=w2_sb[:], in_=w2_in[:])
            nc.sync.dma_start(out=wl_sb[:], in_=wl_in[:])
            nc.sync.dma_start(out=b1_sb[:], in_=b1_in[:])
            nc.sync.dma_start(out=b2_sb[:], in_=b2_in[:])
            nc.sync.dma_start(out=bl_sb[:], in_=bl_in[:])
            nc.sync.dma_start(out=iota_sb[:], in_=iota_in[:])
            nc.sync.dma_start(out=iotar_sb[:], in_=iotar_in[:])
            nc.sync.dma_start(out=cnt_sb[:], in_=cnt_in[:])
            nc.sync.dma_start(out=idf_sb[:], in_=idf_in[:])
            nc.sync.dma_start(out=idb_sb[:], in_=idb_in[:])

            # ---- collective tables (rows are 128 bf16 = 256B for dma_gather;
            #      T2 uses cols 0:64, the rest is never read) ----
            t1_loc = dp.tile([n_loc, P], _BF16)
            t1_full = dp.tile([n_pad, P], _BF16)
            t2_loc = dp.tile([n_loc, P], _BF16)
            t2_full = dp.tile([n_pad, P], _BF16)

            def gather_window(table, w):
                """Issue the window's gathers; return list of (tile, ch0, n).
                Tail subgathers carry trailing -1 pads; their valid count is
                loaded from SBUF so pad descriptors are never generated."""
                tiles = []
                si = 0
                for ch0, capr, col0, lo in [(0, CAP_LO, 0, 0),
                                            (CAP_LO, CAP_HI, CAP_LO * 8, half)]:
                    for g0 in range(0, capr, MAXCH):
                        g1 = min(g0 + MAXCH, capr)
                        gt = gp.tile([P, MAXCH, P], _BF16, tag="g")
                        if rr[0] < 16:
                            # first pass through the 16 slots: zero-fill so
                            # trimmed (never-written) positions stay finite;
                            # later reuses inherit finite data (WAW-ordered)
                            nc.vector.memset(gt[:], 0.0)
                        nreg = cnt_regs[rr[0] % 8]
                        nc.gpsimd.reg_load(
                            nreg, cnt_sb[0:1, w * 16 + si:w * 16 + si + 1])
                        nc.gpsimd.dma_gather(
                            out_ap=gt[:, 0:g1 - g0, :],
                            in_ap=table[lo:lo + half, :],
                            idxs_ap=idx_sb[:, w, col0 + g0 * 8:col0 + g1 * 8],
                            num_idxs=(g1 - g0) * P, num_idxs_reg=nreg,
                            elem_size=P, queue_num=rr[0] % NQ)
                        rr[0] += 1
                        si += 1
                        tiles.append((gt, ch0 + g0, g1 - g0))
                return tiles

            def one_hot(w):
                # One instruction per window: contiguous iota_rep vs
                # broadcast dst column (only one stride-0 operand).
                S = wp.tile([P, CAP, P], _BF16, tag="S")
                nc.vector.tensor_tensor(
                    out=S[:],
                    in0=iotar_sb[:].rearrange("p (c j) -> p c j", c=CAP),
                    in1=dst_sb[:, w * CAP:(w + 1) * CAP, :].to_broadcast([P, CAP, P]),
                    op=mybir.AluOpType.is_equal,
                )
                return S

            def reduce_window(tiles, S, fdim):
                wps = ps.tile([P, fdim], _F32, tag="wp")
                n_mm = sum(n for _, _, n in tiles)
                i = 0
                for gt, ch0, n in tiles:
                    for c in range(n):
                        nc.tensor.matmul(out=wps[:], lhsT=S[:, ch0 + c, :],
                                         rhs=gt[:, c, 0:fdim],
                                         start=(i == 0), stop=(i == n_mm - 1))
                        i += 1
                return wps

            # ---- phase A: T1 = dinv * (x @ W1), local shard ----
            for w in range(W):
                x_t = wp.tile([P, F_IN], _F32, tag="xt")
                nc.sync.dma_start(out=x_t[:], in_=x_in[w * P:(w + 1) * P, :])
                x_bf = wp.tile([P, F_IN], _BF16, tag="xbf")
                nc.vector.tensor_copy(out=x_bf[:], in_=x_t[:])
                xT = wp.tile([P, KIN, P], _BF16, tag="xT")
                for k in range(KIN):
                    trp = ps.tile([P, P], _BF16, tag="tr")
                    nc.tensor.transpose(out=trp[:], in_=x_bf[:, k * P:(k + 1) * P],
                                        identity=idb_sb[:])
                    nc.vector.tensor_copy(out=xT[:, k, :], in_=trp[:])
                hp = ps.tile([P, F_HID], _F32, tag="mm")
                for k in range(KIN):
                    nc.tensor.matmul(out=hp[:], lhsT=xT[:, k, :],
                                     rhs=w1_sb[:, k * F_HID:(k + 1) * F_HID],
                                     start=(k == 0), stop=(k == KIN - 1))
                nc.vector.tensor_scalar(
                    out=t1res_sb[:, w * F_HID:(w + 1) * F_HID], in0=hp[:],
                    scalar1=dinv_sb[:, w:w + 1], scalar2=None,
                    op0=mybir.AluOpType.mult)
                nc.sync.dma_start(out=t1_loc[w * P:(w + 1) * P, :],
                                  in_=t1res_sb[:, w * F_HID:(w + 1) * F_HID])

            # ---- phase B: AllGather T1 ----
            nc.gpsimd.collective_compute(
                "AllGather", mybir.AluOpType.bypass,
                replica_groups=[list(range(N_CORES))],
                ins=[t1_loc.opt()], outs=[t1_full.opt()],
            )

            # ---- phase C: L1 aggregation + relu; build h1T ----
            for w in range(W):
                tiles = gather_window(t1_full, w)
                S = one_hot(w)
                wps = reduce_window(tiles, S, F_HID)
                h1_t = wp.tile([P, F_HID], _F32, tag="h1")
                # h1 = dinv*(agg + t1_local) + b1 ; then relu
                nc.vector.tensor_add(out=h1_t[:], in0=wps[:],
                                     in1=t1res_sb[:, w * F_HID:(w + 1) * F_HID])
                nc.vector.tensor_scalar(out=h1_t[:], in0=h1_t[:],
                                        scalar1=dinv_sb[:, w:w + 1], scalar2=None,
                                        op0=mybir.AluOpType.mult)
                nc.vector.tensor_add(out=h1_t[:], in0=h1_t[:], in1=b1_sb[:])
                h1_bf = wp.tile([P, F_HID], _BF16, tag="h1b")
                nc.scalar.activation(out=h1_bf[:], in_=h1_t[:],
                                     func=mybir.ActivationFunctionType.Relu)
                if debug_taps and w == 0:
                    nc.sync.dma_start(out=taps["h10"][:], in_=h1_t[:])
                trp = ps.tile([P, P], _BF16, tag="tr")
                nc.tensor.transpose(out=trp[:], in_=h1_bf[:], identity=idb_sb[:])
                nc.vector.tensor_copy(out=h1T_sb[:, w * P:(w + 1) * P], in_=trp[:])

            # ---- phase D: T2 = dinv * (h1 @ W2), local shard ----
            for w in range(W):
                t2p = ps.tile([P, F_OUT], _F32, tag="mm")
                nc.tensor.matmul(out=t2p[:], lhsT=h1T_sb[:, w * P:(w + 1) * P],
                                 rhs=w2_sb[:], start=True, stop=True)
                nc.vector.tensor_scalar(
                    out=t2res_sb[:, w * F_OUT:(w + 1) * F_OUT], in0=t2p[:],
                    scalar1=dinv_sb[:, w:w + 1], scalar2=None,
                    op0=mybir.AluOpType.mult)
                t2_t = wp.tile([P, P], _BF16, tag="t2")
                nc.vector.memset(t2_t[:, F_OUT:P], 0.0)
                nc.vector.tensor_copy(out=t2_t[:, 0:F_OUT],
                                      in_=t2res_sb[:, w * F_OUT:(w + 1) * F_OUT])
                nc.sync.dma_start(out=t2_loc[w * P:(w + 1) * P, :], in_=t2_t[:])

            # ---- phase E: AllGather T2 ----
            nc.gpsimd.collective_compute(
                "AllGather", mybir.AluOpType.bypass,
                replica_groups=[list(range(N_CORES))],
                ins=[t2_loc.opt()], outs=[t2_full.opt()],
            )

            # ---- phase F: L2 aggregation + final linear + log_softmax ----
            for w in range(W):
                tiles = gather_window(t2_full, w)
                S = one_hot(w)
                wps = reduce_window(tiles, S, F_OUT)
                h2_t = wp.tile([P, F_OUT], _F32, tag="h2")
                nc.vector.tensor_add(out=h2_t[:], in0=wps[:],
                                     in1=t2res_sb[:, w * F_OUT:(w + 1) * F_OUT])
                nc.vector.tensor_scalar(out=h2_t[:], in0=h2_t[:],
                                        scalar1=dinv_sb[:, w:w + 1], scalar2=None,
                                        op0=mybir.AluOpType.mult)
                nc.vector.tensor_add(out=h2_t[:], in0=h2_t[:], in1=b2_sb[:])
                # final linear: logits = h2 @ Wl + bl (f32)
                h2tp = ps.tile([F_OUT, P], _F32, tag="tr")
                nc.tensor.transpose(out=h2tp[:], in_=h2_t[:], identity=idf_sb[:])
                h2T = wp.tile([F_OUT, P], _F32, tag="h2T")
                nc.vector.tensor_copy(out=h2T[:], in_=h2tp[:])
                lp = ps.tile([P, F_OUT], _F32, tag="mm")
                nc.tensor.matmul(out=lp[:], lhsT=h2T[:], rhs=wl_sb[:],
                                 start=True, stop=True)
                lg = wp.tile([P, F_OUT], _F32, tag="lg")
                nc.vector.tensor_add(out=lg[:], in0=lp[:], in1=bl_sb[:])
                # log_softmax over the free dim
                negmax = wp.tile([P, 1], _F32, tag="nm")
                nc.vector.tensor_reduce(out=negmax[:], in_=lg[:],
                                        axis=mybir.AxisListType.X,
                                        op=mybir.AluOpType.max, negate=True)
                ex = wp.tile([P, F_OUT], _F32, tag="ex")
                sm = wp.tile([P, 1], _F32, tag="sm")
                nc.scalar.activation(out=ex[:], in_=lg[:],
                                     func=mybir.ActivationFunctionType.Exp,
                                     bias=negmax[:], scale=1.0, accum_out=sm[:])
                ls = wp.tile([P, 1], _F32, tag="ls")
                nc.scalar.activation(out=ls[:], in_=sm[:],
                                     func=mybir.ActivationFunctionType.Ln)
                o_t = wp.tile([P, F_OUT], _F32, tag="ot")
                nc.vector.tensor_scalar(out=o_t[:], in0=lg[:],
                                        scalar1=negmax[:], scalar2=ls[:],
                                        op0=mybir.AluOpType.add,
                                        op1=mybir.AluOpType.subtract)
                nc.sync.dma_start(out=out_ext[w * P:(w + 1) * P, :], in_=o_t[:])

    nc.compile()
    return nc


def preprocess(x, edge_index, W1, b1, W2, b2, Wl, bl):
    """Host-side sharding: sort edges by (dst window, src half), pack chunks.
    Self-loops are NOT packed as edges; they are applied densely on-device."""
    n = x.shape[0]
    src = np.asarray(edge_index[0], dtype=np.int64)
    dst = np.asarray(edge_index[1], dtype=np.int64)

    deg = np.bincount(dst, minlength=n).astype(np.float64) + 1.0  # + self loop
    dinv = 1.0 / np.sqrt(deg)

    W = int(np.ceil(n / (N_CORES * P)))
    n_loc = W * P
    n_pad = n_loc * N_CORES
    half = n_pad // 2

    hi = (src >= half).astype(np.int64)
    order = np.argsort((dst // P) * 2 + hi, kind="stable")
    s_src = src[order]
    s_dst = dst[order]
    s_hi = hi[order]

    n_windows = N_CORES * W
    group = (s_dst // P) * 2 + s_hi
    g_counts = np.bincount(group, minlength=2 * n_windows)
    g_starts = np.concatenate([[0], np.cumsum(g_counts)[:-1]])
    j = np.arange(len(s_src)) - g_starts[group]   # rank within group

    CAP_LO = int(np.ceil(g_counts[0::2].max() / P))
    CAP_HI = int(np.ceil(g_counts[1::2].max() / P))
    CAP = CAP_LO + CAP_HI

    gw = s_dst // P
    core = gw // W
    lw = gw % W
    slot = j + s_hi * (CAP_LO * P)                # slot within the window
    p_slot = slot % P
    ch = slot // P

    # dma_gather index streams: index i of a region -> [16r + i%16, i//16]
    idx16 = np.where(s_hi == 1, s_src - half, s_src).astype(np.int16)
    idx_tmp = np.full((N_CORES, 16, W, CAP * 8), -1, dtype=np.int16)  # pads: -1
    idx_tmp[core, j % 16, lw, s_hi * (CAP_LO * 8) + j // 16] = idx16

    # per-subgather valid counts (trailing -1s are trimmed by the ucode; an
    # empty subgather keeps one guard index so the instruction stays legal)
    counts_lo = g_counts[0::2].reshape(N_CORES, W)
    counts_hi = g_counts[1::2].reshape(N_CORES, W)
    MAXCH = 8
    cnt = np.zeros((N_CORES, 1, W * 16), dtype=np.int32)
    si = 0
    for capr, cnts, col0 in [(CAP_LO, counts_lo, 0), (CAP_HI, counts_hi, CAP_LO * 8)]:
        for g0 in range(0, capr, MAXCH):
            g1 = min(g0 + MAXCH, capr)
            v = np.clip(cnts - g0 * P, 1, (g1 - g0) * P)  # [cores, W]
            cnt[:, 0, si::16] = v[:, :W]
            empty = cnts <= g0 * P                        # guard idx at pos g0*128
            ec, ew = np.nonzero(empty)
            idx_tmp[ec, 0, ew, col0 + g0 * 8] = 0
            si += 1
    idx_arr = np.tile(idx_tmp, (1, 8, 1, 1))

    dst_arr = np.full((N_CORES, P, W * CAP, 1), -1.0, dtype=np.float32)
    dst_arr[core, p_slot, lw * CAP + ch, 0] = (s_dst % P).astype(np.float32)

    x_pad = np.zeros((n_pad, F_IN), dtype=np.float32)
    x_pad[:n] = np.asarray(x, dtype=np.float32)
    dinv_pad = np.zeros(n_pad, dtype=np.float32)
    dinv_pad[:n] = dinv

    bf16 = ml_dtypes.bfloat16
    w1_c = np.ascontiguousarray(
        np.asarray(W1, np.float32).reshape(KIN, P, F_HID).transpose(1, 0, 2)
        .reshape(P, KIN * F_HID)).astype(bf16)
    w2_c = np.asarray(W2, np.float32).astype(bf16)
    wl_c = np.asarray(Wl, np.float32)
    b1_r = np.broadcast_to(np.asarray(b1, np.float32), (P, F_HID)).copy()
    b2_r = np.broadcast_to(np.asarray(b2, np.float32), (P, F_OUT)).copy()
    bl_r = np.broadcast_to(np.asarray(bl, np.float32), (P, F_OUT)).copy()
    iota = np.broadcast_to(np.arange(P, dtype=np.float32), (P, 1, P)).astype(bf16).copy()
    iotar = np.broadcast_to(np.tile(np.arange(P, dtype=np.float32), CAP), (P, CAP * P)).astype(bf16).copy()
    idf = np.eye(P, dtype=np.float32)
    idb = np.eye(P, dtype=np.float32).astype(bf16)

    in_maps = []
    for c in range(N_CORES):
        dv = dinv_pad[c * n_loc:(c + 1) * n_loc].reshape(W, P).T.copy()
        in_maps.append({
            "x": x_pad[c * n_loc:(c + 1) * n_loc],
            "idx": idx_arr[c],
            "dst": dst_arr[c],
            "dinv": np.ascontiguousarray(dv),
            "w1": w1_c, "w2": w2_c, "wl": wl_c,
            "b1r": b1_r, "b2r": b2_r, "blr": bl_r,
            "iota": iota, "idf": idf, "idb": idb,
            "iotar": iotar, "cnt": cnt[c],
        })
    return in_maps, (W, CAP_LO, CAP_HI), n, n_loc


def kernel(x, edge_index, W1, b1, W2, b2, Wl, bl):
    from concourse.bass_utils import run_bass_kernel_spmd

    in_maps, key, n, n_loc = preprocess(x, edge_index, W1, b1, W2, b2, Wl, bl)
    if key not in _PROGRAM_CACHE:
        _PROGRAM_CACHE[key] = build_program(*key)
    nc = _PROGRAM_CACHE[key]
    res = run_bass_kernel_spmd(nc, in_maps, list(range(N_CORES)))
    out = np.concatenate([res.results[c]["out"] for c in range(N_CORES)], axis=0)
    return out[:n].astype(np.float32)
